# revision 1
# baseline (speedup 1.0000x reference)
"""Bahdanau encoder-decoder LSTM on 8 Trainium2 NeuronCores.

Strategy: data-parallel over batch (B=32 -> 4 rows per core), all
weights replicated, zero collectives. Each core runs the full
encoder recurrence (T=512 steps) and decoder recurrence (512 steps)
for its 4 batch rows entirely out of SBUF.

Numerics: all matmuls are bf16 x bf16 with fp32 PSUM accumulation
(validated ~2e-3 scale-relative absmax vs the fp32 reference).
Gate/tanh/softmax arithmetic is fp32 on ACT/DVE. Sigmoid is computed
as 0.5*tanh(x/2)+0.5 so the whole kernel uses one ACT table set
(exp_and_others: tanh + exp). The LSTM h state is kept doubled
(H2 = 2h) so the sigmoid affine folds into fused DVE ops; the 0.5 is
restored in the PE transpose (0.5*identity) and on the host for the
DMA'd output.
"""
import numpy as np
import ml_dtypes

import concourse.bass as bass
import concourse.tile as tile_mod
from concourse import mybir
from concourse.bass_utils import run_bass_kernel_spmd
from concourse.tile import TileContext
from concourse.vector_clock import ScopedClock

F32 = mybir.dt.float32
BF16 = mybir.dt.bfloat16
AF = mybir.ActivationFunctionType
OP = mybir.AluOpType
bf16 = ml_dtypes.bfloat16

F, HE, HD, A = 128, 512, 512, 256
B, T = 32, 512
NCORES = 8
BL = B // NCORES  # 4 batch rows per core

# ----------------------------------------------------------------------
# Toolchain workarounds: this walrus build refuses any TPB instruction
# carrying more than one semaphore wait. Hoist extras onto standalone
# EventSemaphore instructions (engine program order keeps semantics).
_TPB_ENGINES = None


def _tpb_engines():
    global _TPB_ENGINES
    if _TPB_ENGINES is None:
        _TPB_ENGINES = {
            mybir.EngineType.PE,
            mybir.EngineType.DVE,
            mybir.EngineType.Activation,
            mybir.EngineType.Pool,
            mybir.EngineType.SP,
        }
    return _TPB_ENGINES


def split_multi_waits(nc, cap=1):
    fn = nc.m.functions[0]
    engines = _tpb_engines()
    for bb in fn.blocks:
        insts = bb.instructions
        out = []
        changed = False
        for inst in insts:
            si = inst.sync_info
            waits = None if si is None else si.on_wait
            if waits is not None and len(waits) > cap and inst.engine in engines:
                extra = list(waits[cap:])
                si.on_wait = list(waits[:cap])
                for j, w in enumerate(extra):
                    ev = mybir.InstEventSemaphore(
                        name=f"{inst.name}-xw{j}", ins=[], outs=[]
                    )
                    ev.engine = inst.engine
                    ev.sync_info = mybir.SyncInfo(on_wait=[w], on_update=[])
                    out.append(ev)
                changed = True
            out.append(inst)
        if changed:
            bb.instructions = out


_patched = False


def patch_tile_drain():
    """Same walrus limitation for the Tile tail drain."""
    global _patched
    if _patched:
        return
    _patched = True

    def _drain(self, tick_clock, wait_clock):
        drain_inst = self.nc.sync.drain()
        wait_clock.add_sem_waits(
            drain_inst.ins, ScopedClock({None: tick_clock.global_clock})
        )
        si = drain_inst.ins.sync_info
        if si is not None and si.on_wait is not None and len(si.on_wait) > 1:
            extra = list(si.on_wait[1:])
            si.on_wait = [si.on_wait[0]]
            for w in extra:
                n2 = self.nc.sync.nop()
                n2.ins.sync_info = mybir.SyncInfo(on_wait=[w], on_update=[])
        self.nc.all_engine_barrier()
        popped = self.nc._tile_sem_poison_stack.pop()
        assert popped is self._sem_poison
        self.nc.clear_and_free_semaphores(list(self.sems.allocated().values()))
        self.nc.all_engine_barrier()

    tile_mod.TileContext._drain_and_barrier = _drain


# ----------------------------------------------------------------------
def build_nc(nt=T, dbg=False):
    """Build the single-core Bass program (SPMD across 8 cores)."""
    patch_tile_drain()
    nc = bass.Bass("TRN2", target_bir_lowering=False, debug=False)
    dbg_outs = {}

    def tap(name, ap_or_tile, shape):
        if not dbg:
            return
        d = nc.dram_tensor("dbg_" + name, list(shape), ap_or_tile.dtype,
                           kind="ExternalOutput")
        nc.sync.dma_start(out=d.ap(), in_=ap_or_tile)
        dbg_outs[name] = d

    # ---- DRAM parameters (per-core inputs prepared on the host) ----
    d_xT = nc.dram_tensor("xT", [128, nt * BL], BF16, kind="ExternalInput")
    d_whhT = nc.dram_tensor("whhT", [4, 128, 4 * HE], BF16, kind="ExternalInput")
    d_wihT = nc.dram_tensor("wihT", [128, 4 * HE], BF16, kind="ExternalInput")
    d_encb = nc.dram_tensor("encb", [1, 4 * HE], BF16, kind="ExternalInput")
    d_dhr = nc.dram_tensor("dhr", [4, 128, 4 * HD + A], BF16, kind="ExternalInput")
    d_dcr = nc.dram_tensor("dcr", [4, 128, 4 * HD], BF16, kind="ExternalInput")
    d_decb = nc.dram_tensor("decb", [1, 4 * HD + A], BF16, kind="ExternalInput")
    d_wencT = nc.dram_tensor("wencT", [4, 128, A], BF16, kind="ExternalInput")
    d_epb = nc.dram_tensor("epb", [2, 128, 1], F32, kind="ExternalInput")
    d_vblk = nc.dram_tensor("vblk", [128, 8 * BL], BF16, kind="ExternalInput")
    d_ones = nc.dram_tensor("ones", [1, BL], BF16, kind="ExternalInput")
    d_id4 = nc.dram_tensor("id4", [BL, BL], F32, kind="ExternalInput")
    d_half4 = nc.dram_tensor("half4", [BL, BL], F32, kind="ExternalInput")
    d_id128 = nc.dram_tensor("id128", [128, 128], BF16, kind="ExternalInput")
    d_hd0T = nc.dram_tensor("hd0T", [128, 4 * BL], BF16, kind="ExternalInput")
    d_cd0 = nc.dram_tensor("cd0", [BL, HD], F32, kind="ExternalInput")
    d_out = nc.dram_tensor("out", [BL, nt, HD], F32, kind="ExternalOutput")

    G = 4 * HE          # 2048 gate width
    GD = 4 * HD + A     # 2304 h-pass width (gates + dscore)

    from contextlib import ExitStack
    with TileContext(nc) as tc, ExitStack() as ctx:
        const = ctx.enter_context(tc.tile_pool(name="const", bufs=1))
        state = ctx.enter_context(tc.tile_pool(name="state", bufs=1))
        work = ctx.enter_context(tc.tile_pool(name="work", bufs=1))

        # ---- load constants into SBUF ----
        xT = const.tile([128, nt * BL], BF16)
        nc.sync.dma_start(out=xT, in_=d_xT.ap())
        whhT = const.tile([128, 4 * G], BF16)
        for k in range(4):
            nc.sync.dma_start(out=whhT[:, k * G:(k + 1) * G], in_=d_whhT.ap()[k])
        wihT = const.tile([128, G], BF16)
        nc.sync.dma_start(out=wihT, in_=d_wihT.ap())
        encb = const.tile([1, G], BF16)
        nc.sync.dma_start(out=encb, in_=d_encb.ap())
        dhr = const.tile([128, 4 * GD], BF16)
        for k in range(4):
            nc.sync.dma_start(out=dhr[:, k * GD:(k + 1) * GD], in_=d_dhr.ap()[k])
        dcr = const.tile([128, 4 * G], BF16)
        for k in range(4):
            nc.sync.dma_start(out=dcr[:, k * G:(k + 1) * G], in_=d_dcr.ap()[k])
        decb = const.tile([1, GD], BF16)
        nc.sync.dma_start(out=decb, in_=d_decb.ap())
        wencT = const.tile([128, 4 * A], BF16)
        for k in range(4):
            nc.sync.dma_start(out=wencT[:, k * A:(k + 1) * A], in_=d_wencT.ap()[k])
        epb = const.tile([128, 2], F32)
        for k in range(2):
            nc.sync.dma_start(out=epb[:, k:k + 1], in_=d_epb.ap()[k])
        vblk = const.tile([128, 8 * BL], BF16)
        nc.sync.dma_start(out=vblk, in_=d_vblk.ap())
        ones = const.tile([1, BL], BF16)
        nc.sync.dma_start(out=ones, in_=d_ones.ap())
        id4 = const.tile([BL, BL], F32)
        nc.sync.dma_start(out=id4, in_=d_id4.ap())
        half4 = const.tile([BL, BL], F32)
        nc.sync.dma_start(out=half4, in_=d_half4.ap())
        id128 = const.tile([128, 128], BF16)
        nc.sync.dma_start(out=id128, in_=d_id128.ap())

        # ---- persistent state ----
        encT = state.tile([128, 4 * nt * BL], BF16)   # [e, (ec, t, b)]
        c_enc = state.tile([BL, HE], F32)
        nc.vector.memset(c_enc, 0.0)
        hdT = state.tile([128, 4 * BL], BF16)
        nc.sync.dma_start(out=hdT, in_=d_hd0T.ap())
        cd = state.tile([BL, HD], F32)
        nc.sync.dma_start(out=cd, in_=d_cd0.ap())
        # 16 block-diag lhsT chunks of [128, BL]; width padded to 80 so the
        # stride-17 scatter slices stay in-bounds.
        wTblk = state.tile([128, 16 * BL + 16], BF16)
        nc.vector.memset(wTblk, 0.0)
        ctxT = state.tile([128, 4 * BL], BF16)

        # ---- per-step work tiles (reused every step) ----
        t_i = work.tile([BL, HE], F32)
        t_f = work.tile([BL, HE], F32)
        t_g = work.tile([BL, HE], F32)
        t_o = work.tile([BL, HE], F32)
        fgate = work.tile([BL, HE], F32)
        fc = work.tile([BL, HE], F32)
        t2 = work.tile([BL, HE], F32)
        th = work.tile([BL, HE], F32)
        H2 = work.tile([BL, HE], F32)
        ds_s = work.tile([BL, A], F32)
        dsT = work.tile([128, 2 * BL], F32)
        w_u = work.tile([BL, T], F32)
        denom = work.tile([BL, 1], F32)
        rcp = work.tile([BL, 1], F32)
        ctx_s = work.tile([BL, HE], F32)

        def lstm_tail(z_ps, c_st, gate_w):
            """Gates + state update from psum z (batch-on-partitions).

            z layout: [BL, 4*gate_w] = i|f|g|o. Updates c_st in place,
            produces H2 (= 2*h) in fp32.
            """
            nc.scalar.activation(out=t_i, in_=z_ps[:, 0:gate_w], func=AF.Tanh,
                                 bias=0.0, scale=0.5)
            nc.scalar.activation(out=t_f, in_=z_ps[:, gate_w:2 * gate_w],
                                 func=AF.Tanh, bias=0.0, scale=0.5)
            nc.scalar.activation(out=t_g, in_=z_ps[:, 2 * gate_w:3 * gate_w],
                                 func=AF.Tanh, bias=0.0, scale=1.0)
            nc.scalar.activation(out=t_o, in_=z_ps[:, 3 * gate_w:4 * gate_w],
                                 func=AF.Tanh, bias=0.0, scale=0.5)
            # f = 0.5*tf + 0.5 ; fc = f*c
            nc.vector.tensor_scalar(out=fgate, in0=t_f, scalar1=0.5, scalar2=0.5,
                                    op0=OP.mult, op1=OP.add)
            nc.vector.tensor_tensor(out=fc, in0=fgate, in1=c_st, op=OP.mult)
            # t2 = (ti + 1) * tg = 2*i*g ; c = 0.5*t2 + fc
            nc.vector.scalar_tensor_tensor(out=t2, in0=t_i, scalar=1.0, in1=t_g,
                                           op0=OP.add, op1=OP.mult)
            nc.vector.scalar_tensor_tensor(out=c_st, in0=t2, scalar=0.5, in1=fc,
                                           op0=OP.mult, op1=OP.add)
            nc.scalar.activation(out=th, in_=c_st, func=AF.Tanh, bias=0.0, scale=1.0)
            # H2 = (to + 1) * th = 2*h
            nc.vector.scalar_tensor_tensor(out=H2, in0=t_o, scalar=1.0, in1=th,
                                           op0=OP.add, op1=OP.mult)

        # ================= ENCODER =================
        with tc.tile_pool(name="eps", bufs=1, space="PSUM") as eps, \
             tc.tile_pool(name="ept", bufs=2, space="PSUM") as ept:
            for t in range(nt):
                z_ps = eps.tile([BL, G], F32, tag="z")
                for ng in range(4):
                    sl = slice(ng * HE, (ng + 1) * HE)
                    mi_l = 0
                    if t > 0:
                        for kc in range(4):
                            nc.tensor.matmul(
                                z_ps[:, sl],
                                encT[:, kc * (nt * BL) + BL * (t - 1):
                                     kc * (nt * BL) + BL * t],
                                whhT[:, kc * G + ng * HE: kc * G + (ng + 1) * HE],
                                start=(mi_l == 0), stop=False)
                            mi_l += 1
                    nc.tensor.matmul(z_ps[:, sl], xT[:, BL * t: BL * (t + 1)],
                                     wihT[:, sl], start=(mi_l == 0), stop=False)
                    nc.tensor.matmul(z_ps[:, sl], ones, encb[:, sl],
                                     start=False, stop=True)
                lstm_tail(z_ps, c_enc, HE)
                # hT: transpose H2 -> psum; scale by 0.5 in the bf16 copy
                # (hardware transpose ignores the identity's values)
                for ec in range(4):
                    pt = ept.tile([128, BL], F32, tag="pt")
                    nc.tensor.transpose(pt, H2[:, ec * 128:(ec + 1) * 128], id4)
                    nc.vector.tensor_scalar(
                        out=encT[:, ec * (nt * BL) + BL * t:
                                 ec * (nt * BL) + BL * (t + 1)],
                        in0=pt, scalar1=0.5, scalar2=None, op0=OP.mult)
                if t == 0 and dbg:
                    zc = work.tile([BL, G], F32, tag="dbg_z")
                    nc.vector.tensor_copy(zc, z_ps)
                    tap("enc_z0", zc, [BL, G])
                    tap("enc_H2_0", H2, [BL, HE])
            tap("encT", encT, [128, 4 * nt * BL])

        # ================= PHASE 2: enc_projT and enc_tb =================
        enc_projT = state.tile([128, 2 * nt * BL], F32)  # [a, (ac, b, t)]
        enc_tb = state.tile([128, 16 * HE], BF16)        # [t', (b, tc, e)]
        with tc.tile_pool(name="p2a", bufs=4, space="PSUM") as p2a, \
             tc.tile_pool(name="p2b", bufs=4, space="PSUM") as p2b:
            for ac in range(2):
                for b in range(BL):
                    pp = p2a.tile([128, nt], F32, tag="pp")
                    for ec in range(4):
                        # rhs: encT e-chunk, batch b, all t (stride BL)
                        base = ec * (nt * BL) + b
                        rhs = encT[:, base: base + BL * (nt - 1) + 1: BL]
                        nc.tensor.matmul(
                            pp,
                            wencT[:, ec * A + ac * 128: ec * A + (ac + 1) * 128],
                            rhs, start=(ec == 0), stop=(ec == 3))
                    # add per-A bias during psum->sbuf copy
                    nc.scalar.activation(
                        out=enc_projT[:, ac * (nt * BL) + b * nt:
                                      ac * (nt * BL) + (b + 1) * nt],
                        in_=pp, func=AF.Identity, bias=epb[:, ac:ac + 1], scale=1.0)
            # enc_tb via PE transposes of encT (bf16 in, f32 psum out is not
            # allowed; transpose keeps dtype, so transpose bf16->bf16 psum)
            for b in range(BL):
                for tc_i in range(nt // 128):
                    for ec in range(4):
                        base = ec * (nt * BL) + (128 * tc_i) * BL + b
                        src = encT[:, base: base + BL * 127 + 1: BL]
                        pt2 = p2b.tile([128, 128], BF16, tag="pt2")
                        nc.tensor.transpose(pt2, src, id128)
                        nc.vector.tensor_copy(
                            enc_tb[:, (b * (nt // 128) + tc_i) * HE + ec * 128:
                                   (b * (nt // 128) + tc_i) * HE + (ec + 1) * 128],
                            pt2)

        tap("enc_projT", enc_projT, [128, 2 * nt * BL])
        tap("enc_tb", enc_tb, [128, 16 * HE])

        # ================= DECODER =================
        align = state.tile([128, 2 * BL * T], BF16)  # [a, (ac, b, t)]
        with tc.tile_pool(name="dz", bufs=1, space="PSUM") as dz, \
             tc.tile_pool(name="dd", bufs=1, space="PSUM") as dd, \
             tc.tile_pool(name="dlc", bufs=1, space="PSUM") as dlc, \
             tc.tile_pool(name="dpt", bufs=2, space="PSUM") as dpt:
            for t in range(nt):
                z_ps = dz.tile([BL, G], F32, tag="zd")
                ds_ps = dd.tile([BL, A], F32, tag="ds")
                # --- h-pass: z(gates) + dscore + bias ---
                for ng in range(4):
                    sl = slice(ng * HD, (ng + 1) * HD)
                    for kc in range(4):
                        nc.tensor.matmul(
                            z_ps[:, sl],
                            hdT[:, kc * BL:(kc + 1) * BL],
                            dhr[:, kc * GD + ng * HD: kc * GD + (ng + 1) * HD],
                            start=(kc == 0), stop=False)
                    nc.tensor.matmul(z_ps[:, sl], ones, decb[:, sl],
                                     start=False, stop=False)
                for kc in range(4):
                    nc.tensor.matmul(
                        ds_ps, hdT[:, kc * BL:(kc + 1) * BL],
                        dhr[:, kc * GD + 4 * HD: (kc + 1) * GD],
                        start=(kc == 0), stop=(kc == 3))
                # --- dscoreT (for tanh bias) ---
                nc.vector.tensor_copy(ds_s, ds_ps)
                for c2 in range(2):
                    pt = dpt.tile([128, BL], F32, tag="ptd")
                    nc.tensor.transpose(pt, ds_s[:, c2 * 128:(c2 + 1) * 128], id4)
                    nc.vector.tensor_copy(dsT[:, c2 * BL:(c2 + 1) * BL], pt)
                # --- attention tanh ---
                for ac in range(2):
                    for b in range(BL):
                        nc.scalar.activation(
                            out=align[:, ac * (BL * T) + b * T: ac * (BL * T) + b * T + nt],
                            in_=enc_projT[:, ac * (nt * BL) + b * nt:
                                          ac * (nt * BL) + (b + 1) * nt],
                            func=AF.Tanh, bias=dsT[:, ac * BL + b: ac * BL + b + 1],
                            scale=1.0)
                # --- V reduction (block-diag) -> logits [BL, nt] ---
                lg_ps = dlc.tile([BL, T], F32, tag="lc")
                q = 0
                for b in range(BL):
                    for ac in range(2):
                        nc.tensor.matmul(
                            lg_ps[:, :nt],
                            vblk[:, (b * 2 + ac) * BL:(b * 2 + ac + 1) * BL],
                            align[:, ac * (BL * T) + b * T: ac * (BL * T) + b * T + nt],
                            start=(q == 0), stop=(q == 7))
                        q += 1
                # --- softmax (no max-sub; logits bounded) ---
                nc.scalar.activation(out=w_u[:, :nt], in_=lg_ps[:, :nt], func=AF.Exp,
                                     bias=0.0, scale=1.0, accum_out=denom)
                nc.vector.reciprocal(rcp, denom)
                # --- wT into block-diag positions ---
                for tc_i in range(nt // 128):
                    pt = dpt.tile([128, BL], F32, tag="ptd")
                    nc.tensor.transpose(pt, w_u[:, tc_i * 128:(tc_i + 1) * 128], id4)
                    # scatter column b to block-diag position 4*(4b+tc)+b
                    stp = 4 * BL + 1
                    dst = wTblk[:, BL * tc_i: BL * tc_i + stp * (BL - 1) + 1: stp]
                    nc.vector.tensor_copy(dst, pt)
                # --- ctx (unnormalized) ---
                ctx_ps = dlc.tile([BL, T], F32, tag="lc")
                q = 0
                nq = BL * (nt // 128)
                for b in range(BL):
                    for tc_i in range(nt // 128):
                        nc.tensor.matmul(
                            ctx_ps[:, :HE],
                            wTblk[:, (b * 4 + tc_i) * BL:(b * 4 + tc_i + 1) * BL],
                            enc_tb[:, (b * (nt // 128) + tc_i) * HE:
                                   (b * (nt // 128) + tc_i + 1) * HE],
                            start=(q == 0), stop=(q == nq - 1))
                        q += 1
                # --- normalize + transpose into ctxT ---
                nc.vector.tensor_scalar(out=ctx_s, in0=ctx_ps[:, :HE], scalar1=rcp,
                                        scalar2=None, op0=OP.mult)
                for ec in range(4):
                    pt = dpt.tile([128, BL], F32, tag="ptd")
                    nc.tensor.transpose(pt, ctx_s[:, ec * 128:(ec + 1) * 128], id4)
                    nc.vector.tensor_copy(ctxT[:, ec * BL:(ec + 1) * BL], pt)
                # --- cell ctx-pass (accumulate into z) ---
                for ng in range(4):
                    sl = slice(ng * HD, (ng + 1) * HD)
                    for kc in range(4):
                        nc.tensor.matmul(
                            z_ps[:, sl],
                            ctxT[:, kc * BL:(kc + 1) * BL],
                            dcr[:, kc * G + ng * HD: kc * G + (ng + 1) * HD],
                            start=False, stop=(kc == 3))
                # --- gates + state ---
                lstm_tail(z_ps, cd, HD)
                # --- hdT for next step (0.5 folded into the copy) ---
                for ec in range(4):
                    pt = dpt.tile([128, BL], F32, tag="ptd")
                    nc.tensor.transpose(pt, H2[:, ec * 128:(ec + 1) * 128], id4)
                    nc.vector.tensor_scalar(
                        out=hdT[:, ec * BL:(ec + 1) * BL],
                        in0=pt, scalar1=0.5, scalar2=None, op0=OP.mult)
                # --- output h (as 2h; host rescales) ---
                nc.sync.dma_start(out=d_out.ap()[:, t, :], in_=H2)
                if t == 0:
                    tap("d_ds0", ds_s, [BL, A])
                    tap("d_dsT0", dsT, [128, 2 * BL])
                    tap("d_align0", align, [128, 2 * BL * T])
                    tap("d_wu0", w_u, [BL, T])
                    tap("d_denom0", denom, [BL, 1])
                    tap("d_ctxs0", ctx_s, [BL, HE])
                    tap("d_H20", H2, [BL, HE])
                    tap("d_wTblk0", wTblk, [128, 16 * BL + 16])

    split_multi_waits(nc)
    if dbg:
        nc._dbg_outs = dbg_outs
    return nc


# ----------------------------------------------------------------------
def _sig(x):
    return 1.0 / (1.0 + np.exp(-x))


def prepare_inputs(inputs, nt=T):
    """Host-side weight/layout prep. Returns (shared_dict, per_core_fn)."""
    f32 = np.float32
    enc_Wih = np.asarray(inputs["enc_Wih"], f32)
    enc_Whh = np.asarray(inputs["enc_Whh"], f32)
    enc_bias = np.asarray(inputs["enc_bih"], f32) + np.asarray(inputs["enc_bhh"], f32)
    Wenc_w = np.asarray(inputs["Wenc_w"], f32)
    Wenc_b = np.asarray(inputs["Wenc_b"], f32)
    Wdec_w = np.asarray(inputs["Wdec_w"], f32)
    Wdec_b = np.asarray(inputs["Wdec_b"], f32)
    V_w = np.asarray(inputs["V_w"], f32)
    attn_bias = np.asarray(inputs["attn_bias"], f32)
    dec_Wih = np.asarray(inputs["dec_Wih"], f32)
    dec_Whh = np.asarray(inputs["dec_Whh"], f32)
    dec_bias = np.asarray(inputs["dec_bih"], f32) + np.asarray(inputs["dec_bhh"], f32)

    sh = {}
    sh["whhT"] = np.ascontiguousarray(
        enc_Whh.T.reshape(4, 128, 4 * HE)).astype(bf16)
    sh["wihT"] = np.ascontiguousarray(enc_Wih.T).astype(bf16)
    sh["encb"] = enc_bias.reshape(1, -1).astype(bf16)
    dec_h_w = (dec_Wih[:, HD:] + dec_Whh)           # [2048, 512]
    dhr = np.concatenate([dec_h_w.T, Wdec_w.T], axis=1)  # [512, 2304]
    sh["dhr"] = np.ascontiguousarray(dhr.reshape(4, 128, 4 * HD + A)).astype(bf16)
    sh["dcr"] = np.ascontiguousarray(
        dec_Wih[:, :HD].T.reshape(4, 128, 4 * HD)).astype(bf16)
    decb_full = np.concatenate([dec_bias, np.zeros(A, f32)]).reshape(1, -1)
    sh["decb"] = decb_full.astype(bf16)
    sh["wencT"] = np.ascontiguousarray(
        Wenc_w.T.reshape(4, 128, A)).astype(bf16)
    sh["epb"] = np.ascontiguousarray(
        (Wenc_b + attn_bias + Wdec_b).reshape(2, 128, 1)).astype(f32)
    vb = np.zeros((128, 8 * BL), f32)
    for b in range(BL):
        for ac in range(2):
            vb[:, (b * 2 + ac) * BL + b] = V_w[0, ac * 128:(ac + 1) * 128]
    sh["vblk"] = vb.astype(bf16)
    sh["ones"] = np.ones((1, BL), f32).astype(bf16)
    sh["id4"] = np.eye(BL, dtype=f32)
    sh["half4"] = (0.5 * np.eye(BL)).astype(f32)
    sh["id128"] = np.eye(128, dtype=f32).astype(bf16)
    # decoder init state (reference lines: z0 from biases only)
    i0, f0, g0, o0 = np.split(dec_bias, 4)
    cd0 = _sig(i0) * np.tanh(g0)
    hd0 = _sig(o0) * np.tanh(cd0)
    hd0T = np.zeros((128, 4 * BL), f32)
    for ec in range(4):
        for b in range(BL):
            hd0T[:, ec * BL + b] = hd0[ec * 128:(ec + 1) * 128]
    sh["hd0T"] = hd0T.astype(bf16)
    sh["cd0"] = np.broadcast_to(cd0, (BL, HD)).astype(f32).copy()

    x = np.asarray(inputs["x"], f32)

    def core_inputs(core):
        xc = x[core * BL:(core + 1) * BL, :nt, :]      # [BL, nt, F]
        xT = np.ascontiguousarray(xc.transpose(2, 1, 0).reshape(128, nt * BL))
        m = dict(sh)
        m["xT"] = xT.astype(bf16)
        return m

    return core_inputs


_cache = {}


def kernel(**inputs):
    nt = np.asarray(inputs["x"]).shape[1]
    if nt not in _cache:
        _cache[nt] = build_nc(nt)
    nc = _cache[nt]
    core_inputs = prepare_inputs(inputs, nt)
    in_maps = [core_inputs(c) for c in range(NCORES)]
    res = run_bass_kernel_spmd(nc, in_maps, core_ids=list(range(NCORES)))
    outs = [res.results[c]["out"] for c in range(NCORES)]
    full = np.concatenate(outs, axis=0) * 0.5
    return full.astype(np.float32)

